# revision 1
# baseline (speedup 1.0000x reference)
"""Sliding-window causal attention (window=1024) for B=2,T=2048,H=16,D=128 fp32
on 8 trn2 NeuronCores. Shards the 32 (batch, head) pairs 4-per-core; each core
runs the same Bass/Tile program on its slice.

Per (b,h) the kernel computes S^T = K @ Q^T blockwise over the 9-block sliding
band, exponentiates on the scalar engine, and accumulates both O^T = V^T @ P
and the softmax denominators (ones-matmul, broadcast to all partitions) in
PSUM. Output is normalized pre-transpose, transposed back to [t, d] on the
tensor engine, and DMA'd out as fp32.
"""
import math

import numpy as np

import concourse.bass as bass
import concourse.bacc as bacc
import concourse.mybir as mybir
from concourse import tile
from concourse.bass_utils import run_bass_kernel_spmd

B, T, H, D = 2, 2048, 16, 128
WINDOW = 1024
NCORES = 8
BH = B * H                  # 32 (b,h) pairs
BH_PER_CORE = BH // NCORES  # 4
NT = T // 128               # 16 seq tiles
G = 4                       # q-tiles per group (512 queries)
NG = NT // G
WB = WINDOW // 128          # window in blocks

f32 = mybir.dt.float32
bf16 = mybir.dt.bfloat16
AF = mybir.ActivationFunctionType
ALU = mybir.AluOpType


def band_blocks(g):
    """Key blocks intersecting group g's sliding band, with the trimmed
    q-tile range [t_min, t_max] each block must serve."""
    out = []
    for b in range(max(0, G * g - WB), G * g + G):
        t_min = max(G * g, b)
        t_max = min(G * g + G - 1, b + WB)
        if t_min <= t_max:
            out.append((b, t_min, t_max))
    return out


def build_nc(n_bh=BH_PER_CORE):
    nc = bacc.Bacc()
    q = nc.declare_dram_parameter("q", [n_bh, T, D], f32, isOutput=False)
    k = nc.declare_dram_parameter("k", [n_bh, T, D], f32, isOutput=False)
    v = nc.declare_dram_parameter("v", [n_bh, T, D], f32, isOutput=False)
    o = nc.declare_dram_parameter("o", [n_bh, T, D], f32, isOutput=True)

    scale = 1.0 / math.sqrt(D)

    with tile.TileContext(nc) as tc:
        with (
            tc.tile_pool(name="const", bufs=1) as constp,
            tc.tile_pool(name="io", bufs=3) as iop,
            tc.tile_pool(name="qt", bufs=3) as qtp,
            tc.tile_pool(name="es", bufs=12) as esp,
            tc.tile_pool(name="outp", bufs=2) as outp,
            tc.tile_pool(name="ps_st", bufs=3, space="PSUM") as ps_st,
            tc.tile_pool(name="ps_pv", bufs=2, space="PSUM") as ps_pv,
            tc.tile_pool(name="ps_sum", bufs=1, space="PSUM") as ps_sum,
            tc.tile_pool(name="ps_tr", bufs=2, space="PSUM") as ps_tr,
        ):
            # --- loads: fp32->bf16 cast during DMA (SWDGE). Issued one bh
            # ahead of use so descriptor generation on the in-order gpsimd
            # queue never stalls the band pipeline.
            def issue_loads(bh):
                qb = iop.tile([128, NT, 128], bf16, tag="qb",
                              name=f"qb_{bh}")
                kb = iop.tile([128, NT, 128], bf16, tag="kb",
                              name=f"kb_{bh}")
                vb = iop.tile([128, NT, 128], bf16, tag="vb",
                              name=f"vb_{bh}")
                nc.gpsimd.dma_start(
                    out=qb[:], in_=q[bh].rearrange("(n p) d -> p n d", p=128))
                nc.gpsimd.dma_start(
                    out=kb[:], in_=k[bh].rearrange("(n p) d -> p n d", p=128))
                nc.gpsimd.dma_start(
                    out=vb[:], in_=v[bh].rearrange("(n p) d -> p n d", p=128))
                return qb, kb, vb

            loaded = {0: issue_loads(0)}

            # --- constants: identity / ones / masks, built in f32, cast to bf16
            ones_f = constp.tile([128, 128], f32)
            ident_f = constp.tile([128, 128], f32)
            mdiag_f = constp.tile([128, 128], f32)
            madiag_f = constp.tile([128, 128], f32)
            nc.gpsimd.memset(ones_f[:], 1.0)
            # identity: keep 1 where p == col
            nc.gpsimd.affine_select(
                out=ident_f[:], in_=ones_f[:], compare_op=ALU.is_equal,
                fill=0.0, base=0, channel_multiplier=1, pattern=[[-1, 128]],
            )
            # diag mask (allowed k <= q): keep where col - p >= 0
            nc.gpsimd.affine_select(
                out=mdiag_f[:], in_=ones_f[:], compare_op=ALU.is_ge,
                fill=0.0, base=0, channel_multiplier=-1, pattern=[[1, 128]],
            )
            # anti-diag mask (allowed k > q): keep where p - col - 1 >= 0
            nc.gpsimd.affine_select(
                out=madiag_f[:], in_=ones_f[:], compare_op=ALU.is_ge,
                fill=0.0, base=-1, channel_multiplier=1, pattern=[[-1, 128]],
            )
            ones = constp.tile([128, 128], bf16)
            ident = constp.tile([128, 128], bf16)
            mdiag = constp.tile([128, 128], bf16)
            madiag = constp.tile([128, 128], bf16)
            nc.vector.tensor_copy(ones[:], ones_f[:])
            nc.vector.tensor_copy(ident[:], ident_f[:])
            nc.vector.tensor_copy(mdiag[:], mdiag_f[:])
            nc.vector.tensor_copy(madiag[:], madiag_f[:])

            for bh in range(n_bh):
                if bh + 1 < n_bh:
                    loaded[bh + 1] = issue_loads(bh + 1)
                qb, kb, vb = loaded.pop(bh)

                # --- transpose q, k to [d, t] layout; per-quad tiles keep
                # dependencies fine-grained so band compute starts early
                qt_q = [qtp.tile([128, 4, 128], bf16, tag=f"qt{i}",
                                 name=f"qt{i}_{bh}")
                        for i in range(NT // 4)]
                kt_q = [qtp.tile([128, 4, 128], bf16, tag=f"kt{i}",
                                 name=f"kt{i}_{bh}")
                        for i in range(NT // 4)]
                for quad in range(NT // 4):
                    for src, dsts in ((qb, qt_q), (kb, kt_q)):
                        tr = ps_tr.tile([128, 4, 128], bf16, tag="tr")
                        for i in range(4):
                            t = quad * 4 + i
                            nc.tensor.matmul(
                                tr[:, i, :], src[:, t, :], ident[:],
                                is_transpose=True,
                                start=(i == 0), stop=(i == 3))
                        nc.vector.tensor_copy(dsts[quad][:], tr[:])

                def kt_slice(b):
                    return kt_q[b // 4][:, b % 4, :]

                # --- sliding-band attention, one group (512 queries) at a time
                for g in range(NG):
                    # widest block first so the start=True matmul covers the
                    # whole PSUM bank; the rest accumulate per-element
                    blocks = sorted(
                        band_blocks(g), key=lambda x: x[1] - x[2])
                    pv = ps_pv.tile([128, 512], f32, tag="pv")
                    sm = ps_sum.tile([128, 512], f32, tag="sm")
                    nblk = len(blocks)
                    for idx, (b, t_min, t_max) in enumerate(blocks):
                        width = (t_max - t_min + 1) * 128
                        off = (t_min - G * g) * 128
                        st = ps_st.tile([128, 512], f32, tag="st")
                        nc.tensor.matmul(
                            st[:, 0:width], kt_slice(b),
                            qt_q[g][:, t_min - G * g:t_max + 1 - G * g, :],
                            start=True, stop=True)
                        es = esp.tile([128, 512], bf16, tag="es")
                        nc.scalar.activation(
                            es[:, 0:width], st[:, 0:width], AF.Exp, scale=scale)
                        # split masks across DVE and the otherwise-idle GpSimd
                        if b >= G * g:
                            nc.vector.tensor_mul(
                                es[:, 0:128], es[:, 0:128], mdiag[:])
                        if b + WB <= G * g + G - 1:
                            nc.gpsimd.tensor_mul(
                                es[:, width - 128:width],
                                es[:, width - 128:width], madiag[:])
                        first = idx == 0
                        last = idx == nblk - 1
                        nc.tensor.matmul(
                            pv[:, off:off + width], vb[:, b, :],
                            es[:, 0:width], start=first, stop=last)
                        nc.tensor.matmul(
                            sm[:, off:off + width], ones[:],
                            es[:, 0:width], start=first, stop=last)

                    # --- normalize + transpose back to [t, d]
                    # transpose the (partition-broadcast) sums so the
                    # reciprocal runs on 4 elements/lane instead of 512
                    smsb = outp.tile([128, 512], f32, tag="smsb")
                    nc.vector.tensor_copy(smsb[:], sm[:])
                    strp = ps_tr.tile([128, 4, 128], f32, tag="tr")
                    for i in range(4):
                        nc.tensor.matmul(
                            strp[:, i, :], smsb[:, 128 * i:128 * (i + 1)],
                            ident_f[:], is_transpose=True,
                            start=(i == 0), stop=(i == 3))
                    recip = outp.tile([128, 4], f32, tag="recip")
                    nc.vector.reciprocal(recip[:], strp[:, :, 0])
                    otn = outp.tile([128, 512], bf16, tag="otn")
                    nc.vector.tensor_copy(otn[:], pv[:])
                    otr = ps_tr.tile([128, 4, 128], bf16, tag="tr")
                    for i in range(4):
                        nc.tensor.matmul(
                            otr[:, i, :], otn[:, 128 * i:128 * (i + 1)],
                            ident[:], is_transpose=True,
                            start=(i == 0), stop=(i == 3))
                    oo = outp.tile([128, 4, 128], f32, tag="oo")
                    for i in range(4):
                        nc.vector.tensor_scalar_mul(
                            oo[:, i, :], otr[:, i, :], recip[:, i:i + 1])
                    nc.sync.dma_start(
                        out=o[bh, 512 * g:512 * (g + 1), :].rearrange(
                            "(t p) d -> p t d", p=128),
                        in_=oo[:])
    if not nc.is_finalized():
        nc.finalize()
    return nc


_nc = None


def _get_nc():
    global _nc
    if _nc is None:
        _nc = build_nc()
    return _nc


def make_in_maps(q, k, v):
    q = np.ascontiguousarray(np.asarray(q, dtype=np.float32))
    k = np.ascontiguousarray(np.asarray(k, dtype=np.float32))
    v = np.ascontiguousarray(np.asarray(v, dtype=np.float32))
    # [B, T, H, D] -> [B*H, T, D]
    qs = np.ascontiguousarray(q.transpose(0, 2, 1, 3).reshape(BH, T, D))
    ks = np.ascontiguousarray(k.transpose(0, 2, 1, 3).reshape(BH, T, D))
    vs = np.ascontiguousarray(v.transpose(0, 2, 1, 3).reshape(BH, T, D))
    return [
        {
            "q": qs[c * BH_PER_CORE:(c + 1) * BH_PER_CORE],
            "k": ks[c * BH_PER_CORE:(c + 1) * BH_PER_CORE],
            "v": vs[c * BH_PER_CORE:(c + 1) * BH_PER_CORE],
        }
        for c in range(NCORES)
    ]


def assemble_out(results):
    out = np.empty((BH, T, D), np.float32)
    for c in range(NCORES):
        out[c * BH_PER_CORE:(c + 1) * BH_PER_CORE] = results[c]["o"]
    return np.ascontiguousarray(
        out.reshape(B, H, T, D).transpose(0, 2, 1, 3))


def kernel(q, k, v, window_size):
    assert int(window_size) == WINDOW
    in_maps = make_in_maps(q, k, v)
    res = run_bass_kernel_spmd(_get_nc(), in_maps, list(range(NCORES))).results
    return assemble_out(res)



# revision 2
# speedup vs baseline: 1.5087x; 1.5087x over previous
"""Sliding-window causal attention (window=1024) for B=2,T=2048,H=16,D=128
on 8 trn2 NeuronCores. Shards the 32 (batch, head) pairs 4-per-core.

v2 layout: inputs are converted to fp16 on the host. q and k are loaded
pre-transposed ([d, t]) straight from HBM via the xbar DMA-transpose engine,
so the tensor engine runs only the band matmuls: S^T = K @ Q^T per 128x128
block pair into a double-bank PSUM tile, one wide exp on the scalar engine,
then PV and the ones-matmul softmax denominators accumulate in PSUM.
Normalization happens pre-transpose with a broadcast reciprocal of the
denominator bank (reciprocal_approx_fast), and the normalized O^T goes back
to [t, d] through the xbar as well. Output is fp16, upcast on the host.
"""
import math

import numpy as np
import ml_dtypes

import concourse.bass as bass
import concourse.bacc as bacc
import concourse.mybir as mybir
from concourse import tile
from concourse.bass_utils import run_bass_kernel_spmd

B, T, H, D = 2, 2048, 16, 128
WINDOW = 1024
NCORES = 8
BH = B * H                  # 32 (b,h) pairs
BH_PER_CORE = BH // NCORES  # 4
NT = T // 128               # 16 seq tiles
G = 4                       # q-tiles per group (512 queries)
NG = NT // G
WB = WINDOW // 128          # window in blocks

f32 = mybir.dt.float32
f16 = mybir.dt.float16
AF = mybir.ActivationFunctionType
ALU = mybir.AluOpType


def band_blocks(g):
    """Key blocks intersecting group g's sliding band, with the trimmed
    q-tile range [t_min, t_max] each block must serve."""
    out = []
    for b in range(max(0, G * g - WB), G * g + G):
        t_min = max(G * g, b)
        t_max = min(G * g + G - 1, b + WB)
        if t_min <= t_max:
            out.append((b, t_min, t_max))
    return out


def build_nc(n_bh=BH_PER_CORE):
    nc = bacc.Bacc()
    q = nc.declare_dram_parameter("q", [n_bh, T, D], f16, isOutput=False)
    k = nc.declare_dram_parameter("k", [n_bh, T, D], f16, isOutput=False)
    v = nc.declare_dram_parameter("v", [n_bh, T, D], f16, isOutput=False)
    o = nc.declare_dram_parameter("o", [n_bh, T, D], f16, isOutput=True)

    scale = 1.0 / math.sqrt(D)

    with tile.TileContext(nc) as tc:
        with (
            tc.tile_pool(name="const", bufs=1) as constp,
            tc.tile_pool(name="io", bufs=2) as iop,
            tc.tile_pool(name="es", bufs=6) as esp,
            tc.tile_pool(name="outp", bufs=2) as outp,
            tc.tile_pool(name="ps_st", bufs=2, space="PSUM") as ps_st,
            tc.tile_pool(name="ps_pv", bufs=2, space="PSUM") as ps_pv,
            tc.tile_pool(name="ps_sum", bufs=2, space="PSUM") as ps_sum,
        ):
            # --- loads: q/k arrive [d, t] via xbar transpose; v natural.
            def prefetch(bh):
                qt = iop.tile([128, T], f16, tag="qt", name=f"qt_{bh}")
                kt = iop.tile([128, T], f16, tag="kt", name=f"kt_{bh}")
                vb = iop.tile([128, NT, 128], f16, tag="vb", name=f"vb_{bh}")
                nc.sync.dma_start_transpose(out=kt[:], in_=k[bh])
                nc.sync.dma_start_transpose(out=qt[:], in_=q[bh])
                nc.sync.dma_start(
                    out=vb[:], in_=v[bh].rearrange("(n p) d -> p n d", p=128))
                return qt, kt, vb

            loaded = {0: prefetch(0)}

            # --- constants: ones for the denominator matmul, causal masks
            ones_f = constp.tile([128, 128], f32)
            mdiag_f = constp.tile([128, 128], f32)
            madiag_f = constp.tile([128, 128], f32)
            nc.gpsimd.memset(ones_f[:], 1.0)
            # diag mask (allowed k <= q): keep where col - p >= 0
            nc.gpsimd.affine_select(
                out=mdiag_f[:], in_=ones_f[:], compare_op=ALU.is_ge,
                fill=0.0, base=0, channel_multiplier=-1, pattern=[[1, 128]],
            )
            # anti-diag mask (allowed k > q): keep where p - col - 1 >= 0
            nc.gpsimd.affine_select(
                out=madiag_f[:], in_=ones_f[:], compare_op=ALU.is_ge,
                fill=0.0, base=-1, channel_multiplier=1, pattern=[[-1, 128]],
            )
            ones = constp.tile([128, 128], f16)
            mdiag = constp.tile([128, 128], f16)
            madiag = constp.tile([128, 128], f16)
            nc.vector.tensor_copy(ones[:], ones_f[:])
            nc.vector.tensor_copy(mdiag[:], mdiag_f[:])
            nc.vector.tensor_copy(madiag[:], madiag_f[:])

            for bh in range(n_bh):
                qt, kt, vb = loaded.pop(bh)

                for g in range(NG):
                    blocks = sorted(band_blocks(g), key=lambda x: x[1] - x[2])
                    n = len(blocks)
                    # pair widest with narrowest: first slot is always a
                    # full 512-wide block at PSUM offset 0
                    pairs = [(blocks[i], blocks[n - 1 - i])
                             for i in range(n // 2)]
                    pv = ps_pv.tile([128, 512], f32, tag="pv")
                    sm = ps_sum.tile([128, 512], f32, tag="sm")

                    def emit_pvsm(pair_idx, es, pair):
                        first = pair_idx == 0
                        last = pair_idx == len(pairs) - 1
                        for j, (b, t_min, t_max) in enumerate(pair):
                            w = (t_max - t_min + 1) * 128
                            off = (t_min - G * g) * 128
                            eo = 512 * j
                            nc.tensor.matmul(
                                pv[:, off:off + w], vb[:, b, :],
                                es[:, eo:eo + w],
                                start=first and j == 0, stop=last and j == 1)
                            nc.tensor.matmul(
                                sm[:, off:off + w], ones[:],
                                es[:, eo:eo + w],
                                start=first and j == 0, stop=last and j == 1)

                    pending = None
                    for pi, pair in enumerate(pairs):
                        stp = ps_st.tile([128, 1024], f32, tag="st")
                        es = esp.tile([128, 1024], f16, tag="es")
                        w1 = 0
                        for j, (b, t_min, t_max) in enumerate(pair):
                            w = (t_max - t_min + 1) * 128
                            nc.tensor.matmul(
                                stp[:, 512 * j:512 * j + w],
                                kt[:, 128 * b:128 * b + 128],
                                qt[:, 128 * t_min:128 * (t_max + 1)],
                                start=True, stop=True)
                            if j == 1:
                                w1 = w
                        # one exp across both blocks (the [w0, 512) gap is
                        # garbage but never read downstream)
                        nc.scalar.activation(
                            es[:, 0:512 + w1], stp[:, 0:512 + w1], AF.Exp,
                            scale=scale)
                        # causal trim masks on the band edges
                        for j, (b, t_min, t_max) in enumerate(pair):
                            w = (t_max - t_min + 1) * 128
                            eo = 512 * j
                            if b >= G * g:
                                nc.vector.tensor_mul(
                                    es[:, eo:eo + 128], es[:, eo:eo + 128],
                                    mdiag[:])
                            if b + WB <= G * g + G - 1:
                                nc.gpsimd.tensor_mul(
                                    es[:, eo + w - 128:eo + w],
                                    es[:, eo + w - 128:eo + w], madiag[:])
                        if pending is not None:
                            emit_pvsm(pending[0], pending[1], pending[2])
                        pending = (pi, es, pair)
                    emit_pvsm(pending[0], pending[1], pending[2])

                    # --- normalize pre-transpose with broadcast reciprocal,
                    # then xbar back to [t, d] and store
                    rec = outp.tile([128, 512], f32, tag="rec")
                    nc.vector.reciprocal_approx_fast(rec[:], sm[:])
                    otn = outp.tile([128, 512], f16, tag="otn")
                    nc.vector.tensor_mul(otn[:], pv[:], rec[:])
                    oot = outp.tile([128, G, 128], f16, tag="oot")
                    nc.sync.dma_start_transpose(out=oot[:], in_=otn[:])
                    nc.sync.dma_start(
                        out=o[bh, 512 * g:512 * (g + 1), :].rearrange(
                            "(t p) d -> p t d", p=128),
                        in_=oot[:])

                    # prefetch next bh once the first group is in flight
                    if g == 0 and bh + 1 < n_bh:
                        loaded[bh + 1] = prefetch(bh + 1)
    if not nc.is_finalized():
        nc.finalize()
    return nc


_nc = None


def _get_nc():
    global _nc
    if _nc is None:
        _nc = build_nc()
    return _nc


def make_in_maps(q, k, v):
    # [B, T, H, D] -> [B*H, T, D], fp16 (converted host-side)
    qs = np.ascontiguousarray(
        np.asarray(q, dtype=np.float32).transpose(0, 2, 1, 3)
        .reshape(BH, T, D)).astype(np.float16)
    ks = np.ascontiguousarray(
        np.asarray(k, dtype=np.float32).transpose(0, 2, 1, 3)
        .reshape(BH, T, D)).astype(np.float16)
    vs = np.ascontiguousarray(
        np.asarray(v, dtype=np.float32).transpose(0, 2, 1, 3)
        .reshape(BH, T, D)).astype(np.float16)
    return [
        {
            "q": qs[c * BH_PER_CORE:(c + 1) * BH_PER_CORE],
            "k": ks[c * BH_PER_CORE:(c + 1) * BH_PER_CORE],
            "v": vs[c * BH_PER_CORE:(c + 1) * BH_PER_CORE],
        }
        for c in range(NCORES)
    ]


def assemble_out(results):
    out = np.empty((BH, T, D), np.float32)
    for c in range(NCORES):
        out[c * BH_PER_CORE:(c + 1) * BH_PER_CORE] = np.asarray(
            results[c]["o"], dtype=np.float32)
    return np.ascontiguousarray(
        out.reshape(B, H, T, D).transpose(0, 2, 1, 3))


def kernel(q, k, v, window_size):
    assert int(window_size) == WINDOW
    in_maps = make_in_maps(q, k, v)
    res = run_bass_kernel_spmd(_get_nc(), in_maps, list(range(NCORES))).results
    return assemble_out(res)
